# revision 1
# baseline (speedup 1.0000x reference)
"""NetVLAD-style vq_codebook kernel for 8 Trainium2 NeuronCores.

Reference computation (per full input):
  assn = BN(x @ clusters); softmax over 80 clusters, drop 16 ghosts
  vlad[b,d,k] = sum_n assn[b,n,k] x[b,n,d] - a_sum[b,k]*clusters2[d,k]
  intra-normalize over d, flatten, global L2 normalize -> (B, D*K)

Sharding: data-parallel over batch B (B/8 batches per core). BatchNorm
statistics (sum and sum-of-squares per cluster column) are all-reduced
across the 8 cores (2*80 floats). Everything else is local.

Implementation notes:
 - x is cast to fp16 on load (gpsimd cast-DMA), kept in natural layout
   (token-partition) for the vlad matmul, and transposed on-chip with the
   DMA XBAR transpose into d-partition layout for the assignment matmul.
 - PE matmuls: per token tile 4 accumulating (128x128fp16)@(128x80fp16)
   matmuls for cluster assignment; vlad: per token tile one
   (128x64)@(128x512) matmul accumulating vlad^T = (64k, 512d) per batch,
   plus an N=1 matmul against a ones column for a_sum.
 - BN stats via PE: ones-column stationary matmuls against assn and
   assn^2 accumulate per-column sums in PSUM.
 - softmax without max-subtraction (logits are exactly BN-normalized,
   |logit| <~ 6, exp is safe in fp32).
"""

import sys

for _p in ("/opt/trn_rl_repo", "/root/.axon_site/_ro/trn_rl_repo"):
    if _p not in sys.path:
        sys.path.insert(0, _p)

import numpy as np

import concourse.bacc as bacc
import concourse.mybir as mybir
import concourse.tile as tile
from concourse.bass_utils import run_bass_kernel_spmd

F32 = mybir.dt.float32
F16 = mybir.dt.float16
AX = mybir.AxisListType
OP = mybir.AluOpType
ACTF = mybir.ActivationFunctionType

N_CORES = 8
D = 512
KG = 80          # clusters + ghosts
K = 64           # real clusters
N_SEQ = 2048
TPB = N_SEQ // 128   # token tiles per batch = 16
BN_EPS = 1e-5
L2_EPS = 1e-12


def build(b_loc=4, n_cores=N_CORES, with_collective=True):
    """Build the per-core program. b_loc = batches per core."""
    nt = b_loc * TPB                # token tiles per core
    tok = nt * 128                  # tokens per core
    total_tok = tok * n_cores       # global token count for BN stats

    nc = bacc.Bacc("TRN2", target_bir_lowering=False, debug=False,
                   dynamic_dma_scratch_size=65536)

    x = nc.declare_dram_parameter("x", [tok, D], F32, isOutput=False)
    cl = nc.declare_dram_parameter("clusters", [D, KG], F32, isOutput=False)
    c2 = nc.declare_dram_parameter("clusters2", [D, K], F32, isOutput=False)
    gam = nc.declare_dram_parameter("bn_gamma", [1, KG], F32, isOutput=False)
    bet = nc.declare_dram_parameter("bn_beta", [1, KG], F32, isOutput=False)
    y = nc.declare_dram_parameter("y", [b_loc, D * K], F32, isOutput=True)

    ones_row_c = nc.inline_tensor(np.ones((1, 128), np.float32), name="c_ones_row")

    with tile.TileContext(nc) as tc:
        with (
            tc.tile_pool(name="persist", bufs=1) as persist,
            tc.tile_pool(name="work", bufs=4) as work,
            tc.tile_pool(name="dram", bufs=1, space="DRAM") as dram,
        ):
            # ---- persistent SBUF tensors ----
            xh = persist.tile([128, nt, D], F16, name="xh")
            ones16 = persist.tile([128, 1], F16, name="ones16")
            assn = persist.tile([128, nt, KG], F16, name="assn")
            asqP = persist.tile([128, nt, KG], F16, name="asqP")
            sm = persist.tile([128, nt, K], F16, name="sm")
            clh = persist.tile([128, 4, KG], F16, name="clh")
            c2n = persist.tile([128, 4, K], F32, name="c2n")
            ones_row = persist.tile([1, 128], F32, name="ones_row")
            gamma = persist.tile([1, KG], F32, name="gamma")
            beta = persist.tile([1, KG], F32, name="beta")
            ss = persist.tile([1, 2 * KG], F32, name="ss")
            stats_sb = persist.tile([1, 2 * KG], F32, name="stats_sb")
            stats_g = persist.tile([1, 2 * KG], F32, name="stats_g")
            bcB = persist.tile([128, 2 * KG], F16, name="bcB")

            stats_in = dram.tile([1, 2 * KG], F32, name="stats_in")
            stats_out = dram.tile([1, 2 * KG], F32, name="stats_out")

            # ---- phase 0: constants + x load/cast ----
            nc.sync.dma_start(ones_row[:], ones_row_c.ap()[:, :])
            nc.sync.dma_start(gamma[:], gam[:, :])
            nc.sync.dma_start(beta[:], bet[:, :])
            # clusters -> fp16 chunks (cast dma): chunk c partition p = row 128c+p
            nc.gpsimd.dma_start(
                clh[:], cl.ap().rearrange("(c p) k -> p c k", p=128))
            # clusters2 natural layout; PE-transposed to (64k, 512d) below
            nc.sync.dma_start(
                c2n[:], c2.ap().rearrange("(c p) k -> p c k", p=128))
            nc.vector.memset(ones16[:], 1.0)

            # x cast-DMA in groups of 8 token tiles (SWDGE casts
            # fp32->fp16 in the DMA engines; HBM read is the real cost)
            xr = x.ap().rearrange("(t p) d -> p t d", p=128)
            for g in range(nt // 8):
                nc.gpsimd.dma_start(
                    xh[:, 8 * g:8 * (g + 1), :], xr[:, 8 * g:8 * (g + 1), :])

            # ---- phases 0b-2: transposes, assignment matmul, BN stats ----
            with tc.tile_pool(name="ps1", bufs=5, space="PSUM") as ps1:
                # BN stats accumulate in their own banks, pipelined one
                # tile-group behind the assignment matmuls (safe: start=True
                # clears has_written per-bank only)
                pstat_s = ps1.tile([1, 4 * KG], F32, name="pstat_s",
                                   tag="st_s", bufs=1)
                pstat_q = ps1.tile([1, 4 * KG], F32, name="pstat_q",
                                   tag="st_q", bufs=1)
                ng = nt // 4

                def emit_stats(g):
                    nc.tensor.matmul(pstat_s[:], ones16[:],
                                     assn[:, 4 * g:4 * g + 4, :],
                                     start=(g == 0), stop=(g == ng - 1),
                                     skip_group_check=True)
                    nc.tensor.matmul(pstat_q[:], ones16[:],
                                     asqP[:, 4 * g:4 * g + 4, :],
                                     start=(g == 0), stop=(g == ng - 1),
                                     skip_group_check=True)

                for tg in range(nt // 8):
                    xhTg = work.tile([128, 32, 128], F16, name="xhTg",
                                     tag="xhT", bufs=4)
                    # batched XBAR transpose: (128, 8*512) -> (128, 32, 128)
                    # with logical row 128*e + p at [:, e, :]; e = 4*j + c,
                    # d = 128*c + p (chunk-major per tile), matching clh
                    nc.sync.dma_start(xhTg[:, :, :],
                                      xh[:, 8 * tg:8 * (tg + 1), :],
                                      transpose=True)
                    for j in range(8):
                        t = 8 * tg + j
                        p1 = ps1.tile([128, KG], F32, name="p1", tag="p1")
                        for c in range(4):
                            nc.tensor.matmul(
                                p1[:], xhTg[:, 4 * j + c, :], clh[:, c, :],
                                start=(c == 0), stop=(c == 3),
                                skip_group_check=True)
                        nc.vector.tensor_copy(assn[:, t, :], p1[:])
                        if t % 4 == 3:
                            nc.scalar.square(asqP[:, t - 3:t + 1, :],
                                             assn[:, t - 3:t + 1, :])
                    if tg >= 1:
                        emit_stats(2 * (tg - 1))
                        emit_stats(2 * (tg - 1) + 1)
                emit_stats(ng - 2)
                emit_stats(ng - 1)


                # ---- phase 2: all-reduce stats ----
                nc.vector.tensor_reduce(
                    stats_sb[:, :KG],
                    pstat_s[:].rearrange("p (t k) -> p k t", t=4),
                    axis=AX.X, op=OP.add)
                nc.vector.tensor_reduce(
                    stats_sb[:, KG:],
                    pstat_q[:].rearrange("p (t k) -> p k t", t=4),
                    axis=AX.X, op=OP.add)

            nc.sync.dma_start(stats_in[:], stats_sb[:])
            if with_collective:
                nc.gpsimd.collective_compute(
                    "AllReduce", OP.add,
                    replica_groups=[list(range(n_cores))],
                    ins=[stats_in.opt()], outs=[stats_out.opt()])
            else:
                nc.sync.dma_start(stats_out[:], stats_in[:])
            nc.sync.dma_start(stats_g[:], stats_out[:])

            t_mean = work.tile([1, KG], F32, name="t_mean", tag="sv", bufs=6)
            t_var = work.tile([1, KG], F32, name="t_var", tag="sv", bufs=6)
            t_sd = work.tile([1, KG], F32, name="t_sd", tag="sv", bufs=6)
            t_rs = work.tile([1, KG], F32, name="t_rs", tag="sv", bufs=6)
            t_ms = work.tile([1, KG], F32, name="t_ms", tag="sv", bufs=6)
            inv_n = 1.0 / float(total_tok)
            nc.vector.tensor_scalar_mul(t_mean[:], stats_g[:, :KG], inv_n)
            nc.vector.tensor_scalar_mul(t_var[:], stats_g[:, KG:], inv_n)
            nc.vector.tensor_tensor(t_ms[:], t_mean[:], t_mean[:], op=OP.mult)
            nc.vector.tensor_tensor(t_var[:], t_var[:], t_ms[:], op=OP.subtract)
            nc.vector.tensor_scalar_add(t_var[:], t_var[:], BN_EPS)
            nc.scalar.sqrt(t_sd[:], t_var[:])
            nc.vector.reciprocal(t_rs[:], t_sd[:])
            nc.vector.tensor_tensor(ss[:, :KG], t_rs[:], gamma[:], op=OP.mult)
            nc.vector.tensor_tensor(t_ms[:], t_mean[:], ss[:, :KG], op=OP.mult)
            nc.vector.tensor_tensor(ss[:, KG:], beta[:], t_ms[:], op=OP.subtract)

            # ---- phases 3-5: softmax (all batches first, one Exp LUT
            # load), vlad matmul with x stationary -> natural (d,k) layout,
            # then normalization (software-pipelined across batches) ----
            with (
                tc.tile_pool(name="ps2", bufs=2, space="PSUM") as ps2,
                tc.tile_pool(name="elem", bufs=4) as elem,
                tc.tile_pool(name="vpost", bufs=3) as vpost,
            ):
                pbc = ps2.tile([128, 2 * KG], F32, name="pbc", tag="bc2")
                nc.tensor.matmul(pbc[:], ones_row[:], ss[:], start=True,
                                 stop=True, skip_group_check=True)
                nc.vector.tensor_copy(bcB[:], pbc[:])
                scale_b = bcB[:, :KG].rearrange("p (a k) -> p a k", a=1)
                shift_b = bcB[:, KG:].rearrange("p (a k) -> p a k", a=1)

                for b in range(b_loc):
                    t0 = b * TPB
                    te = elem.tile([128, TPB, KG], F16, name="te", tag="te")
                    nc.vector.tensor_tensor(
                        te[:], assn[:, t0:t0 + TPB, :],
                        scale_b.to_broadcast([128, TPB, KG]), op=OP.mult)
                    nc.vector.tensor_tensor(
                        te[:], te[:], shift_b.to_broadcast([128, TPB, KG]),
                        op=OP.add)
                    nc.scalar.activation(te[:], te[:], ACTF.Exp)
                    denom = work.tile([128, TPB], F16, name="denom", tag="dn")
                    with nc.allow_low_precision("fp16 softmax denom"):
                        nc.vector.tensor_reduce(denom[:], te[:], axis=AX.X,
                                                op=OP.add)
                    recip = work.tile([128, TPB], F16, name="recip", tag="rc")
                    with nc.allow_low_precision("fp16 softmax recip"):
                        nc.vector.reciprocal(recip[:], denom[:])
                    nc.vector.tensor_tensor(
                        sm[:, t0:t0 + TPB, :], te[:, :, :K],
                        recip[:].rearrange("p (t a) -> p t a", a=1)
                        .to_broadcast([128, TPB, K]), op=OP.mult)

                state = {}

                def mm_stage(b):
                    t0 = b * TPB
                    pv2 = ps2.tile([128, 4 * K], F32, name="pv2", tag="pv")
                    pas = ps2.tile([1, 4 * K], F32, name="pas", tag="pas")
                    pv3 = pv2[:].rearrange("p (c k) -> p c k", c=4)
                    # NOTE: groups must be contiguous per PSUM bank region --
                    # start=True clears has_written for the whole bank, so
                    # interleaving c-groups drops earlier partial sums.
                    for c in range(4):
                        for i in range(TPB):
                            t = t0 + i
                            nc.tensor.matmul(
                                pv3[:, c, :],
                                xh[:, t, c * 128:(c + 1) * 128],
                                sm[:, t, :],
                                start=(i == 0), stop=(i == TPB - 1),
                                skip_group_check=True)
                    for g in range(TPB // 4):
                        nc.tensor.matmul(pas[:], ones16[:],
                                         sm[:, t0 + 4 * g:t0 + 4 * g + 4, :],
                                         start=(g == 0), stop=(g == TPB // 4 - 1),
                                         skip_group_check=True)
                    state[b] = (pv2, pas)

                def post_stage(b):
                    pv2, pas = state.pop(b)
                    pv3 = pv2[:].rearrange("p (c k) -> p c k", c=4)
                    pa_sb = work.tile([1, K], F32, name="pa_sb", tag="pas_sb")
                    nc.vector.tensor_reduce(
                        pa_sb[:], pas[:].rearrange("p (i k) -> p k i", i=4),
                        axis=AX.X, op=OP.add)
                    pamB = ps2.tile([128, K], F32, name="pamB", tag="bc2")
                    nc.tensor.matmul(pamB[:], ones_row[:], pa_sb[:],
                                     start=True, stop=True,
                                     skip_group_check=True)
                    # v = vlad - a_sum*clusters2 in natural (p, c, k) layout
                    av = vpost.tile([128, 4, K], F32, name="av", tag="av")
                    nc.vector.tensor_tensor(
                        av[:], c2n[:],
                        pamB[:].rearrange("p (a k) -> p a k", a=1)
                        .to_broadcast([128, 4, K]), op=OP.mult)
                    v = vpost.tile([128, 4, K], F32, name="v", tag="v")
                    nc.vector.tensor_tensor(v[:], pv3[:], av[:],
                                            op=OP.subtract)
                    # intra-norm over d (partitions x chunks) via PE
                    sq = vpost.tile([128, 4, K], F16, name="sq", tag="sq")
                    with nc.allow_low_precision("fp16 norm squares"):
                        nc.vector.tensor_tensor(sq[:], v[:], v[:], op=OP.mult)
                    pnrm = ps2.tile([1, 4 * K], F32, name="pnrm", tag="pnrm")
                    nc.tensor.matmul(pnrm[:], ones16[:], sq[:], start=True,
                                     stop=True, skip_group_check=True)
                    nrm2 = work.tile([1, K], F32, name="nrm2", tag="nr")
                    nc.vector.tensor_reduce(
                        nrm2[:], pnrm[:].rearrange("p (c k) -> p k c", c=4),
                        axis=AX.X, op=OP.add)
                    snorm = work.tile([1, K], F32, name="snorm", tag="nr")
                    nc.scalar.sqrt(snorm[:], nrm2[:])
                    nc.vector.tensor_scalar_max(snorm[:], snorm[:], L2_EPS)
                    rn = work.tile([1, K], F32, name="rn", tag="nr")
                    nc.vector.reciprocal(rn[:], snorm[:])
                    # global norm: g2 = sum_k (snorm*rn)^2
                    t1 = work.tile([1, K], F32, name="t1", tag="nr")
                    nc.vector.tensor_tensor(t1[:], snorm[:], rn[:], op=OP.mult)
                    nc.vector.tensor_tensor(t1[:], t1[:], t1[:], op=OP.mult)
                    g2 = work.tile([1, 1], F32, name="g2", tag="g1", bufs=6)
                    nc.vector.tensor_reduce(g2[:], t1[:], axis=AX.X, op=OP.add)
                    gs = work.tile([1, 1], F32, name="gs", tag="g1", bufs=6)
                    nc.scalar.sqrt(gs[:], g2[:])
                    nc.vector.tensor_scalar_max(gs[:], gs[:], L2_EPS)
                    gr = work.tile([1, 1], F32, name="gr", tag="g1", bufs=6)
                    nc.vector.reciprocal(gr[:], gs[:])
                    nc.vector.tensor_scalar(rn[:], rn[:], gr[:], None,
                                            op0=OP.mult)
                    prnB = ps2.tile([128, K], F32, name="prnB", tag="bc2")
                    nc.tensor.matmul(prnB[:], ones_row[:], rn[:], start=True,
                                     stop=True, skip_group_check=True)
                    vf = vpost.tile([128, 4, K], F32, name="vf", tag="vf")
                    nc.vector.tensor_tensor(
                        vf[:], v[:],
                        prnB[:].rearrange("p (a k) -> p a k", a=1)
                        .to_broadcast([128, 4, K]), op=OP.mult)
                    yb = y[b, :].rearrange("(c p k) -> p c k", p=128, k=K)
                    nc.sync.dma_start(yb[:, :, :], vf[:])

                for b in range(b_loc):
                    mm_stage(b)
                    if b >= 1:
                        post_stage(b - 1)
                post_stage(b_loc - 1)
    nc.compile()
    return nc


_CACHE = {}


def _get(b_loc, n_cores, with_collective):
    key = (b_loc, n_cores, with_collective)
    if key not in _CACHE:
        _CACHE[key] = build(b_loc, n_cores, with_collective)
    return _CACHE[key]


def make_in_maps(x, clusters, clusters2, bn_gamma, bn_beta, n_cores=N_CORES):
    B = x.shape[0]
    b_loc = B // n_cores
    shared = {
        "clusters": np.ascontiguousarray(clusters, np.float32),
        "clusters2": np.ascontiguousarray(
            np.asarray(clusters2).reshape(D, K), np.float32),
        "bn_gamma": np.ascontiguousarray(
            np.asarray(bn_gamma).reshape(1, KG), np.float32),
        "bn_beta": np.ascontiguousarray(
            np.asarray(bn_beta).reshape(1, KG), np.float32),
    }
    in_maps = []
    for i in range(n_cores):
        m = dict(shared)
        m["x"] = np.ascontiguousarray(
            np.asarray(x[i * b_loc:(i + 1) * b_loc]).reshape(
                b_loc * N_SEQ, D), np.float32)
        in_maps.append(m)
    return in_maps


def kernel(x, clusters, clusters2, bn_gamma, bn_beta):
    B, N, Dd = x.shape
    assert (N, Dd) == (N_SEQ, D) and B % N_CORES == 0
    b_loc = B // N_CORES
    nc = _get(b_loc, N_CORES, True)
    in_maps = make_in_maps(x, clusters, clusters2, bn_gamma, bn_beta)
    res = run_bass_kernel_spmd(nc, in_maps, core_ids=list(range(N_CORES)))
    out = np.concatenate([res.results[i]["y"] for i in range(N_CORES)], axis=0)
    return out



# revision 30
# speedup vs baseline: 1.7220x; 1.7220x over previous
"""NetVLAD-style vq_codebook kernel for 8 Trainium2 NeuronCores.

Reference computation (per full input):
  assn = BN(x @ clusters); softmax over 80 clusters, drop 16 ghosts
  vlad[b,d,k] = sum_n assn[b,n,k] x[b,n,d] - a_sum[b,k]*clusters2[d,k]
  intra-normalize over d, flatten, global L2 normalize -> (B, D*K)

Sharding: data-parallel over batch B (B/8 batches per core). BatchNorm
statistics (sum and sum-of-squares per cluster column) are all-reduced
across the 8 cores (2*80 floats). Everything else is local.

Implementation notes (v2):
 - x cast-loaded fp32->fp16 by the DMA engines (SWDGE) in 4-tile groups,
   kept in natural token-partition layout for the vlad matmul.
 - x transposed on the PE (transpose matmuls via identity, 128 cycles per
   128x128 tile, fp16 PSUM out) instead of the DMA XBAR; PSUM->SBUF
   copies split between DVE and Act to balance.
 - BN sum stats via linearity: sum_n assn = (sum_n x) @ clusters, where
   sum_n x accumulates on the PE with free-size-1 matmuls (~free).
   Sum-of-squares via Act squares + ones-column PE matmuls.
 - rsqrt built as exp(-0.5*ln(v+eps)) so every Act func (Ln/Exp/Square/
   Copy) lives in one activation table: zero table reloads.
 - softmax denominator: fold 80->40 with one fp16 add (2x mode), then
   reduce; one batch's BN-apply runs on gpsimd to offload DVE.
 - global L2 norm is sqrt(K) exactly (K unit-norm columns), folded into
   the intra-norm reciprocal as bias ln(1/sqrt(K)) in the Exp.
 - per-batch vlad in PSUM c-groups (sequential groups per bank), post
   stage pipelined one batch behind the vlad matmuls.
"""

import sys

for _p in ("/opt/trn_rl_repo", "/root/.axon_site/_ro/trn_rl_repo"):
    if _p not in sys.path:
        sys.path.insert(0, _p)

import numpy as np

import concourse.bacc as bacc
import concourse.mybir as mybir
import concourse.tile as tile
from concourse.bass_utils import run_bass_kernel_spmd

F32 = mybir.dt.float32
F16 = mybir.dt.float16
AX = mybir.AxisListType
OP = mybir.AluOpType
ACTF = mybir.ActivationFunctionType

N_CORES = 8
D = 512
KG = 80          # clusters + ghosts
K = 64           # real clusters
N_SEQ = 2048
TPB = N_SEQ // 128   # token tiles per batch = 16
BN_EPS = 1e-5
L2_EPS = 1e-12


def build(b_loc=4, n_cores=N_CORES, with_collective=True):
    """Build the per-core program. b_loc = batches per core.

    During the build we steer the activation-table selector to the one
    act_func_set that contains ALL functions we use (Ln/Exp/Square/Copy:
    'natural_log_exp_and_others'); the default greedy first-match picks
    ln-only and exp-only tables and reloads (1.3us) on every Ln<->Exp
    alternation.  The patched dict is restored right after compile; the
    emitted act_func_set_id is the real json index, so the NEFF is
    unchanged semantically.
    """
    return _build_inner(b_loc, n_cores, with_collective)


def _steer_act_tables(nc):
    from concourse.hw_specs import get_activation_tables
    tabs = get_activation_tables(nc.m.arch)
    saved = {k: set(v) for k, v in tabs.items()}
    keep = "natural_log_exp_and_others"
    if keep in tabs:
        for name in tabs:
            if name != keep:
                tabs[name] = set()
    return tabs, saved


def _restore_act_tables(tabs, saved):
    for k, v in saved.items():
        tabs[k] = v


def _build_inner(b_loc, n_cores, with_collective):
    nt = b_loc * TPB                # token tiles per core
    tok = nt * 128                  # tokens per core
    total_tok = tok * n_cores       # global token count for BN stats

    nc = bacc.Bacc("TRN2", target_bir_lowering=False, debug=False,
                   dynamic_dma_scratch_size=65536)

    x = nc.declare_dram_parameter("x", [tok, D], F32, isOutput=False)
    cl = nc.declare_dram_parameter("clusters", [D, KG], F32, isOutput=False)
    c2 = nc.declare_dram_parameter("clusters2", [D, K], F32, isOutput=False)
    gam = nc.declare_dram_parameter("bn_gamma", [1, KG], F32, isOutput=False)
    bet = nc.declare_dram_parameter("bn_beta", [1, KG], F32, isOutput=False)
    y = nc.declare_dram_parameter("y", [b_loc, D * K], F32, isOutput=True)

    ones_row_c = nc.inline_tensor(np.ones((1, 128), np.float32), name="c_ones_row")
    ident_c = nc.inline_tensor(np.eye(128, dtype=np.float16), name="c_ident")
    identn_c = nc.inline_tensor(-np.eye(64, dtype=np.float16), name="c_identn")

    with tile.TileContext(nc) as tc:
        with (
            tc.tile_pool(name="persist", bufs=1) as persist,
            tc.tile_pool(name="work", bufs=4) as work,
            tc.tile_pool(name="dram", bufs=1, space="DRAM") as dram,
        ):
            # ---- persistent SBUF tensors ----
            xh = persist.tile([128, nt, D], F16, name="xh")
            assn = persist.tile([128, nt, KG], F16, name="assn")
            sm = persist.tile([128, nt, K], F16, name="sm")
            clh = persist.tile([128, 4, KG], F16, name="clh")
            c2h = persist.tile([128, 4, K], F16, name="c2h")
            c2hT = persist.tile([64, 4, 128], F16, name="c2hT")
            identN = persist.tile([64, K], F16, name="identN")
            ident = persist.tile([128, 128], F16, name="ident")
            ones16 = persist.tile([128, 1], F16, name="ones16")
            ones_row = persist.tile([1, 128], F32, name="ones_row")
            ones_row_h = persist.tile([1, 128], F16, name="ones_row_h")
            gamma = persist.tile([1, KG], F32, name="gamma")
            beta = persist.tile([1, KG], F32, name="beta")
            warm16 = persist.tile([128, 16], F16, name="warm16")
            zrow = persist.tile([1, 4 * K], F16, name="zrow")
            c_eps = persist.tile([1, 1], F32, name="c_eps")
            c_lnk = persist.tile([1, 1], F32, name="c_lnk")
            ss = persist.tile([1, 2 * KG], F16, name="ss")
            stats_sb = persist.tile([1, 2 * KG], F32, name="stats_sb")
            stats_g = persist.tile([1, 2 * KG], F32, name="stats_g")
            bcB = persist.tile([128, 2 * KG], F16, name="bcB")

            stats_in = dram.tile([1, 2 * KG], F32, name="stats_in")
            stats_out = dram.tile([1, 2 * KG], F32, name="stats_out")

            # ---- phase 0: constants + x load/cast ----
            nc.vector.memset(warm16[:], 1.0)
            nc.vector.memset(zrow[:], 0.0)
            nc.vector.memset(ones16[:], 1.0)
            nc.vector.memset(ones_row_h[:], 1.0)
            nc.vector.memset(c_eps[:], BN_EPS)
            nc.vector.memset(c_lnk[:], float(-0.5 * np.log(K)))
            # early dummy activation: the single table load (1.3us) hides
            # under the x-load DMA phase
            nc.scalar.square(warm16[:], warm16[:])
            nc.sync.dma_start(ones_row[:], ones_row_c.ap()[:, :])
            nc.sync.dma_start(ident[:], ident_c.ap()[:, :])
            nc.sync.dma_start(identN[:], identn_c.ap()[:, :])
            nc.sync.dma_start(gamma[:], gam[:, :])
            nc.sync.dma_start(beta[:], bet[:, :])
            # x cast-DMA in groups of 4 token tiles (SWDGE casts fp32->fp16
            # in the DMA engines); clusters interleaved after the first two
            # groups so the first transposes start ASAP
            xr = x.ap().rearrange("(t p) d -> p t d", p=128)
            for g in range(2):
                nc.gpsimd.dma_start(
                    xh[:, 4 * g:4 * (g + 1), :], xr[:, 4 * g:4 * (g + 1), :])
            # clusters / clusters2 -> fp16 (cast dma): chunk c part p = row 128c+p
            nc.gpsimd.dma_start(
                clh[:], cl.ap().rearrange("(c p) k -> p c k", p=128))
            nc.gpsimd.dma_start(
                c2h[:], c2.ap().rearrange("(c p) k -> p c k", p=128))
            for g in range(2, nt // 4):
                nc.gpsimd.dma_start(
                    xh[:, 4 * g:4 * (g + 1), :], xr[:, 4 * g:4 * (g + 1), :])

            # ---- phase 1: PE transposes + assignment matmul + BN stats ----
            with tc.tile_pool(name="ps1", bufs=1, space="PSUM") as ps1:
                # every PSUM tag gets full-bank tiles so concurrent
                # accumulation groups never share a bank
                # one open accumulation group per PSUM bank at any time:
                # a start=True wipes other OPEN groups in its bank (probed
                # on hw); stopped data survives.
                psts = ps1.tile([1, 512], F32, name="psts", tag="psts",
                                bufs=1)
                pstq = ps1.tile([1, 512], F32, name="pstq", tag="pstq",
                                bufs=1)

                # warm up the tensor engine p-state clock at t~0 (own
                # region of the psts bank, single-shot)
                nc.tensor.matmul(psts[0:1, 400:416], warm16[:, :1],
                                 warm16[:], start=True, stop=True,
                                 skip_group_check=True)

                # clusters2 transposed to (64k, 4c, 128d) for the fused
                # -a_sum*c2 accumulation into the vlad psum (borrows an
                # xT-pool tile before the pipeline starts)
                c2tp = ps1.tile([128, 2, 4, 128], F16, name="c2tp", tag="xT",
                                bufs=3)
                for c in range(4):
                    nc.tensor.transpose(c2tp[0:64, 0, c, :], c2h[:, c, :],
                                        ident[:])
                nc.vector.tensor_copy(c2hT[:], c2tp[0:64, 0, :, :])

                ng = nt // 4
                DEPTH = 4               # software pipeline depth (tiles)
                p1_groups = {}
                xhT_pairs = {}

                def pair_front(p):
                    """transposes + paired xT copy + xsum matmuls, tiles
                    t=2p, 2p+1."""
                    xp = ps1.tile([128, 2, 4, 128], F16, name="xp", tag="xT",
                                  bufs=3)
                    for j in range(2):
                        t = 2 * p + j
                        for c in range(4):
                            nc.tensor.transpose(
                                xp[:, j, c, :],
                                xh[:, t, 128 * c:128 * (c + 1)], ident[:])

                    xt = work.tile([128, 2, 4, 128], F16, name="xhT",
                                   tag="xhT", bufs=4)
                    xhT_pairs[p] = xt
                    # PSUM->SBUF copies: ~25 pairs on DVE, 7 on Act
                    if p % 5 == 4:
                        nc.scalar.copy(xt[:], xp[:])
                    else:
                        nc.vector.tensor_copy(xt[:], xp[:])

                def tile_back(t):
                    """assignment matmuls for tile t + group stats."""
                    g = t // 4
                    if t % 4 == 0:
                        p1_groups[g] = ps1.tile([128, 4, 128], F32, name="p1",
                                                tag="p1", bufs=2)
                    p1g = p1_groups[g]
                    xt = xhT_pairs[t // 2]
                    for c in range(4):
                        nc.tensor.matmul(
                            p1g[:, t % 4, :KG], xt[:, t % 2, c, :],
                            clh[:, c, :], start=(c == 0), stop=(c == 3),
                            skip_group_check=True)
                    if t % 2 == 1:
                        xhT_pairs.pop(t // 2)
                    if t % 4 == 3:
                        # assn spill on Act; squares on DVE from the fp16
                        # copy (2x mode); column sums of squares on PE
                        nc.scalar.copy(assn[:, t - 3:t + 1, :],
                                       p1g[:, :, :KG])
                        asq = work.tile([128, 4, KG], F16, name="asq",
                                        tag="asq", bufs=3)
                        with nc.allow_low_precision("fp16 squares"):
                            nc.vector.tensor_tensor(
                                asq[:], assn[:, t - 3:t + 1, :],
                                assn[:, t - 3:t + 1, :], op=OP.mult)
                        nc.tensor.matmul(psts[0:1, :4 * KG], ones16[:],
                                         assn[:, t - 3:t + 1, :],
                                         start=(g == 0), stop=(g == ng - 1),
                                         skip_group_check=True)
                        nc.tensor.matmul(pstq[0:1, :4 * KG], ones16[:],
                                         asq[:], start=(g == 0),
                                         stop=(g == ng - 1),
                                         skip_group_check=True)
                        p1_groups.pop(g)

                for p in range(nt // 2):
                    pair_front(p)
                    for j in range(2):
                        t = 2 * p + j - DEPTH
                        if t >= 0:
                            tile_back(t)
                for t in range(nt - DEPTH, nt):
                    tile_back(t)

                # ---- phase 2 head: assemble stats [sum(80) | sumsq(80)] ----
                nc.vector.tensor_reduce(
                    stats_sb[:, :KG],
                    psts[0:1, :4 * KG].rearrange("p (t k) -> p k t", t=4),
                    axis=AX.X, op=OP.add)
                nc.vector.tensor_reduce(
                    stats_sb[:, KG:],
                    pstq[0:1, :4 * KG].rearrange("p (t k) -> p k t", t=4),
                    axis=AX.X, op=OP.add)

            # ---- phase 2: all-reduce stats + BN parameters ----
            nc.sync.dma_start(stats_in[:], stats_sb[:])
            if with_collective:
                nc.gpsimd.collective_compute(
                    "AllReduce", OP.add,
                    replica_groups=[list(range(n_cores))],
                    ins=[stats_in.opt()], outs=[stats_out.opt()])
            else:
                nc.sync.dma_start(stats_out[:], stats_in[:])
            nc.sync.dma_start(stats_g[:], stats_out[:])

            t_mean = work.tile([1, KG], F32, name="t_mean", tag="sv", bufs=6)
            t_msq = work.tile([1, KG], F32, name="t_msq", tag="sv", bufs=6)
            t_var = work.tile([1, KG], F32, name="t_var", tag="sv", bufs=6)
            t_ln = work.tile([1, KG], F32, name="t_ln", tag="sv", bufs=6)
            t_rs = work.tile([1, KG], F32, name="t_rs", tag="sv", bufs=6)
            t_ms = work.tile([1, KG], F32, name="t_ms", tag="sv", bufs=6)
            inv_n = 1.0 / float(total_tok)
            nc.vector.tensor_scalar_mul(t_mean[:], stats_g[:, :KG], inv_n)
            nc.vector.tensor_tensor(t_msq[:], t_mean[:], t_mean[:], op=OP.mult)
            # var = sumsq*inv_n - mean^2 in one fused op
            nc.vector.scalar_tensor_tensor(
                t_var[:], stats_g[:, KG:], inv_n, t_msq[:],
                op0=OP.mult, op1=OP.subtract)
            # rsqrt(var+eps) = exp(-0.5*ln(var+eps)): stays in the ln/exp
            # activation table (no table reload vs Sqrt)
            nc.scalar.activation(t_ln[:], t_var[:], ACTF.Ln, bias=c_eps[:])
            nc.scalar.activation(t_rs[:], t_ln[:], ACTF.Exp, scale=-0.5)
            nc.vector.tensor_tensor(ss[:, :KG], t_rs[:], gamma[:], op=OP.mult)
            nc.vector.tensor_tensor(t_ms[:], t_mean[:], ss[:, :KG], op=OP.mult)
            nc.vector.tensor_tensor(ss[:, KG:], beta[:], t_ms[:], op=OP.subtract)

            # ---- phases 3-5: softmax, vlad matmul, normalization ----
            # Half-batch (8-tile) softmax granularity keeps every in-order
            # engine queue busy and lets the vlad matmuls start early.
            H = TPB // 2
            with (
                tc.tile_pool(name="ps2", bufs=1, space="PSUM") as ps2,
                tc.tile_pool(name="elem", bufs=3) as elem,
                tc.tile_pool(name="vpost", bufs=2) as vpost,
            ):
                pbc = ps2.tile([128, 2 * KG], F32, name="pbc", tag="pbc",
                               bufs=1)
                nc.tensor.matmul(pbc[:], ones_row_h[:], ss[:],
                                 start=True, stop=True, skip_group_check=True)
                nc.vector.tensor_copy(bcB[:], pbc[:])
                scale_b = bcB[:, :KG].rearrange("p (a k) -> p a k", a=1)
                shift_b = bcB[:, KG:].rearrange("p (a k) -> p a k", a=1)

                te_tiles = {}
                state = {}
                # engine per batch: 'v' = DVE, 'p' = gpsimd (stt ops)
                TE_ENG = {0: 'v', 1: 'p', 2: 'v', 3: 'p'}
                SM_ENG = {0: 'v', 1: 'p', 2: 'v', 3: 'v'}

                def softmax_head(b, h):
                    """BN-apply (DVE or Pool) + exp (Act) for half h."""
                    t0 = b * TPB + h * H
                    te = elem.tile([128, H, KG], F16, name="te", tag="te",
                                   bufs=5)
                    te_tiles[(b, h)] = te
                    if TE_ENG[b] == 'v':
                        nc.vector.tensor_tensor(
                            te[:], assn[:, t0:t0 + H, :],
                            scale_b.to_broadcast([128, H, KG]), op=OP.mult)
                        nc.vector.tensor_tensor(
                            te[:], te[:],
                            shift_b.to_broadcast([128, H, KG]), op=OP.add)
                    else:
                        nc.gpsimd.tensor_tensor(
                            te[:], assn[:, t0:t0 + H, :],
                            scale_b.to_broadcast([128, H, KG]), op=OP.mult)
                        nc.gpsimd.tensor_tensor(
                            te[:], te[:],
                            shift_b.to_broadcast([128, H, KG]), op=OP.add)
                    nc.scalar.activation(te[:], te[:], ACTF.Exp)

                def softmax_tail(b, h):
                    """denominator fold+reduce, recip, sm for half h."""
                    t0 = b * TPB + h * H
                    te = te_tiles.pop((b, h))
                    f1 = elem.tile([128, H, KG // 2], F16, name="f1",
                                   tag="f1", bufs=2)
                    with nc.allow_low_precision("fp16 softmax denom"):
                        nc.vector.tensor_tensor(
                            f1[:], te[:, :, :KG // 2], te[:, :, KG // 2:],
                            op=OP.add)
                        denom = work.tile([128, H], F16, name="denom",
                                          tag="dn", bufs=2)
                        nc.vector.tensor_reduce(denom[:], f1[:], axis=AX.X,
                                                op=OP.add)
                        recip = work.tile([128, H], F16, name="recip",
                                          tag="rc", bufs=2)
                        nc.vector.reciprocal(recip[:], denom[:])
                    rb = recip[:].rearrange("p (t a) -> p t a", a=1) \
                        .to_broadcast([128, H, K])
                    if SM_ENG[b] == 'v':
                        nc.vector.tensor_tensor(
                            sm[:, t0:t0 + H, :], te[:, :, :K], rb, op=OP.mult)
                    else:
                        nc.gpsimd.tensor_tensor(
                            sm[:, t0:t0 + H, :], te[:, :, :K], rb, op=OP.mult)

                def mm_seg(b, h):
                    """vlad + a_sum matmul segment for half h; the c-groups
                    stay open until the -a_sum*c2 closer in post_head."""
                    t0 = b * TPB + h * H
                    if h == 0:
                        pv = ps2.tile([128, 4, 128], F32, name="pv", tag="pv",
                                      bufs=3)
                        ppost = ps2.tile([1, 512], F32, name="ppost",
                                         tag="ppost", bufs=2)
                        state[b] = (pv, ppost)
                        # zero the whole bank region with one start matmul
                        # so ONE group stays open; everything after
                        # accumulates with start=False
                        nc.tensor.matmul(
                            pv[:, :, :K], ones_row_h[:], zrow[:],
                            start=True, stop=False, skip_group_check=True)
                    pv, ppost = state[b]
                    pas = ppost[0:1, :K]
                    for c in range(4):
                        for i in range(H):
                            t = t0 + i
                            nc.tensor.matmul(
                                pv[:, c, :K],
                                xh[:, t, c * 128:(c + 1) * 128],
                                sm[:, t, :],
                                start=False, stop=False,
                                skip_group_check=True)
                    for i in range(H):
                        nc.tensor.matmul(pas, ones16[:], sm[:, t0 + i, :],
                                         start=(h == 0 and i == 0),
                                         stop=(h == 1 and i == H - 1),
                                         skip_group_check=True)

                def post_head(b):
                    """a_sum bcast, -a*c2 folded into the vlad psum via a
                    negated-identity matmul, squares, intra-norm sums,
                    rn = exp(-0.5 ln(nrm2) + ln(1/sqrt(K)))."""
                    pv, ppost = state.pop(b)
                    pa_sb = work.tile([1, K], F16, name="pa_sb", tag="pas_sb",
                                      bufs=2)
                    with nc.allow_low_precision("fp16 a_sum"):
                        nc.vector.tensor_copy(pa_sb[:], ppost[0:1, :K])
                    bc = ps2.tile([128, 2 * K], F32, name="bc", tag="bc",
                                  bufs=2)
                    pamB = bc[:, :K]
                    nc.tensor.matmul(pamB, ones_row_h[:], pa_sb[:],
                                     start=True, stop=True,
                                     skip_group_check=True)
                    dgN = vpost.tile([64, K], F16, name="dgN", tag="dgN")
                    nc.vector.tensor_tensor(dgN[:], identN[:], pamB[0:64, :],
                                            op=OP.mult)
                    # close the four c-groups: pv[:,c,:] -= c2[:,k]*a_sum[k]
                    for c in range(4):
                        nc.tensor.matmul(pv[:, c, :K], c2hT[:, c, :], dgN[:],
                                         start=False, stop=True,
                                         skip_group_check=True)
                    sq = vpost.tile([128, 4, K], F16, name="sq", tag="sq")
                    nc.scalar.square(sq[:], pv[:, :, :K])
                    pnrm = ppost[0:1, K:2 * K]
                    for c in range(4):
                        nc.tensor.matmul(pnrm, ones16[:], sq[:, c, :],
                                         start=(c == 0), stop=(c == 3),
                                         skip_group_check=True)
                    # global L2 norm is exactly sqrt(K) (K unit columns),
                    # folded in via the exp bias
                    rn = work.tile([1, K], F16, name="rn", tag="nr", bufs=4)
                    lnn = work.tile([1, K], F32, name="lnn", tag="nr2",
                                    bufs=4)
                    nc.scalar.activation(lnn[:], pnrm, ACTF.Ln, bias=0.0)
                    with nc.allow_low_precision("fp16 norm recip"):
                        nc.scalar.activation(rn[:], lnn[:], ACTF.Exp,
                                             scale=-0.5, bias=c_lnk[:])
                    state[b] = (pv, rn, bc)

                def post_tail(b):
                    pv, rn, bc = state.pop(b)
                    prnB = bc[:, K:]
                    nc.tensor.matmul(prnB, ones_row_h[:], rn[:],
                                     start=True, stop=True,
                                     skip_group_check=True)
                    # stage the broadcast in SBUF: a DVE op may read at most
                    # one PSUM operand, and vf already reads the vlad psum
                    rnB = vpost.tile([128, K], F16, name="rnB", tag="rnB")
                    nc.scalar.copy(rnB[:], prnB)
                    vf = vpost.tile([128, 4, K], F32, name="vf", tag="vf")
                    nc.vector.tensor_tensor(
                        vf[:], pv[:, :, :K],
                        rnB[:].rearrange("p (a k) -> p a k", a=1)
                        .to_broadcast([128, 4, K]), op=OP.mult)
                    yb = y[b, :].rearrange("(c p k) -> p c k", p=128, k=K)
                    nc.sync.dma_start(yb[:, :, :], vf[:])

                # emission: batch 0/2 prepared on DVE, 1/3 on gpsimd; posts
                # pipelined two batches behind
                softmax_head(0, 0)
                softmax_head(0, 1)
                softmax_head(1, 0)
                softmax_head(1, 1)
                for i, b in enumerate(range(b_loc)):
                    for h in range(2):
                        softmax_tail(b, h)
                        mm_seg(b, h)
                    if b + 2 < b_loc:
                        softmax_head(b + 2, 0)
                        softmax_head(b + 2, 1)
                    if b >= 2:
                        post_tail(b - 2)
                    if b >= 1:
                        post_head(b - 1)
                post_tail(b_loc - 2)
                post_head(b_loc - 1)
                post_tail(b_loc - 1)
    tabs, saved = _steer_act_tables(nc)
    try:
        nc.compile()
    finally:
        _restore_act_tables(tabs, saved)
    return nc


_CACHE = {}


def _get(b_loc, n_cores, with_collective):
    key = (b_loc, n_cores, with_collective)
    if key not in _CACHE:
        _CACHE[key] = build(b_loc, n_cores, with_collective)
    return _CACHE[key]


def make_in_maps(x, clusters, clusters2, bn_gamma, bn_beta, n_cores=N_CORES):
    B = x.shape[0]
    b_loc = B // n_cores
    shared = {
        "clusters": np.ascontiguousarray(clusters, np.float32),
        "clusters2": np.ascontiguousarray(
            np.asarray(clusters2).reshape(D, K), np.float32),
        "bn_gamma": np.ascontiguousarray(
            np.asarray(bn_gamma).reshape(1, KG), np.float32),
        "bn_beta": np.ascontiguousarray(
            np.asarray(bn_beta).reshape(1, KG), np.float32),
    }
    in_maps = []
    for i in range(n_cores):
        m = dict(shared)
        m["x"] = np.ascontiguousarray(
            np.asarray(x[i * b_loc:(i + 1) * b_loc]).reshape(
                b_loc * N_SEQ, D), np.float32)
        in_maps.append(m)
    return in_maps


def kernel(x, clusters, clusters2, bn_gamma, bn_beta):
    B, N, Dd = x.shape
    assert (N, Dd) == (N_SEQ, D) and B % N_CORES == 0
    b_loc = B // N_CORES
    nc = _get(b_loc, N_CORES, True)
    in_maps = make_in_maps(x, clusters, clusters2, bn_gamma, bn_beta)
    res = run_bass_kernel_spmd(nc, in_maps, core_ids=list(range(N_CORES)))
    out = np.concatenate([res.results[i]["y"] for i in range(N_CORES)], axis=0)
    return out


# revision 46
# speedup vs baseline: 1.9430x; 1.1283x over previous
"""NetVLAD-style vq_codebook kernel for 8 Trainium2 NeuronCores.

Reference computation (per full input):
  assn = BN(x @ clusters); softmax over 80 clusters, drop 16 ghosts
  vlad[b,d,k] = sum_n assn[b,n,k] x[b,n,d] - a_sum[b,k]*clusters2[d,k]
  intra-normalize over d, flatten, global L2 normalize -> (B, D*K)

Sharding: data-parallel over batch B (B/8 batches per core). BatchNorm
statistics (sum and sum-of-squares per cluster column) are all-reduced
across the 8 cores (2*80 floats). Everything else is local.

Implementation notes (v2):
 - x cast-loaded fp32->fp16 by the DMA engines (SWDGE) in 4-tile groups,
   kept in natural token-partition layout for the vlad matmul.
 - x transposed on the PE (transpose matmuls via identity, 128 cycles per
   128x128 tile, fp16 PSUM out) instead of the DMA XBAR; PSUM->SBUF
   copies split between DVE and Act to balance.
 - BN sum stats via linearity: sum_n assn = (sum_n x) @ clusters, where
   sum_n x accumulates on the PE with free-size-1 matmuls (~free).
   Sum-of-squares via Act squares + ones-column PE matmuls.
 - rsqrt built as exp(-0.5*ln(v+eps)) so every Act func (Ln/Exp/Square/
   Copy) lives in one activation table: zero table reloads.
 - softmax denominator: fold 80->40 with one fp16 add (2x mode), then
   reduce; one batch's BN-apply runs on gpsimd to offload DVE.
 - global L2 norm is sqrt(K) exactly (K unit-norm columns), folded into
   the intra-norm reciprocal as bias ln(1/sqrt(K)) in the Exp.
 - per-batch vlad in PSUM c-groups (sequential groups per bank), post
   stage pipelined one batch behind the vlad matmuls.
"""

import sys

for _p in ("/opt/trn_rl_repo", "/root/.axon_site/_ro/trn_rl_repo"):
    if _p not in sys.path:
        sys.path.insert(0, _p)

import numpy as np

import concourse.bacc as bacc
import concourse.mybir as mybir
import concourse.tile as tile
from concourse.bass_utils import run_bass_kernel_spmd

F32 = mybir.dt.float32
F16 = mybir.dt.float16
AX = mybir.AxisListType
OP = mybir.AluOpType
ACTF = mybir.ActivationFunctionType

N_CORES = 8
D = 512
KG = 80          # clusters + ghosts
K = 64           # real clusters
N_SEQ = 2048
TPB = N_SEQ // 128   # token tiles per batch = 16
BN_EPS = 1e-5
L2_EPS = 1e-12


def build(b_loc=4, n_cores=N_CORES, with_collective=True):
    """Build the per-core program. b_loc = batches per core.

    During the build we steer the activation-table selector to the one
    act_func_set that contains ALL functions we use (Ln/Exp/Square/Copy:
    'natural_log_exp_and_others'); the default greedy first-match picks
    ln-only and exp-only tables and reloads (1.3us) on every Ln<->Exp
    alternation.  The patched dict is restored right after compile; the
    emitted act_func_set_id is the real json index, so the NEFF is
    unchanged semantically.
    """
    return _build_inner(b_loc, n_cores, with_collective)


def _steer_act_tables(nc):
    from concourse.hw_specs import get_activation_tables
    tabs = get_activation_tables(nc.m.arch)
    saved = {k: set(v) for k, v in tabs.items()}
    keep = "natural_log_exp_and_others"
    if keep in tabs:
        for name in tabs:
            if name != keep:
                tabs[name] = set()
    return tabs, saved


def _restore_act_tables(tabs, saved):
    for k, v in saved.items():
        tabs[k] = v


def _build_inner(b_loc, n_cores, with_collective):
    nt = b_loc * TPB                # token tiles per core
    tok = nt * 128                  # tokens per core
    total_tok = tok * n_cores       # global token count for BN stats

    nc = bacc.Bacc("TRN2", target_bir_lowering=False, debug=False,
                   dynamic_dma_scratch_size=65536)

    x = nc.declare_dram_parameter("x", [tok, D], F32, isOutput=False)
    cl = nc.declare_dram_parameter("clusters", [D, KG], F32, isOutput=False)
    c2 = nc.declare_dram_parameter("clusters2", [D, K], F32, isOutput=False)
    gam = nc.declare_dram_parameter("bn_gamma", [1, KG], F32, isOutput=False)
    bet = nc.declare_dram_parameter("bn_beta", [1, KG], F32, isOutput=False)
    y = nc.declare_dram_parameter("y", [b_loc, D * K], F32, isOutput=True)

    _idc = np.zeros((128, 192), np.float16)
    _idc[:, :128] = np.eye(128, dtype=np.float16)
    _idc[:64, 128:] = -np.eye(64, dtype=np.float16)
    ident_c = nc.inline_tensor(_idc, name="c_ident")

    with tile.TileContext(nc) as tc:
        with (
            tc.tile_pool(name="persist", bufs=1) as persist,
            tc.tile_pool(name="work", bufs=4) as work,
            tc.tile_pool(name="dram", bufs=1, space="DRAM") as dram,
        ):
            # ---- persistent SBUF tensors ----
            xh = persist.tile([128, nt, D], F16, name="xh")
            assn = persist.tile([128, nt, KG], F16, name="assn")
            sm = persist.tile([128, nt, K], F16, name="sm")
            clh = persist.tile([128, 4, KG], F16, name="clh")
            c2h = persist.tile([128, 4, K], F16, name="c2h")
            c2hT = persist.tile([64, 4, 128], F16, name="c2hT")
            xsum_sb = persist.tile([128, 4], F16, name="xsum_sb")
            identN = persist.tile([64, K], F16, name="identN")
            identB = persist.tile([128, 192], F16, name="identB")
            ident = identB[:, :128]
            identN = identB[0:64, 128:]
            ones16 = persist.tile([128, 1], F16, name="ones16")
            ones_row_h = persist.tile([1, 128], F16, name="ones_row_h")
            gamma = persist.tile([1, KG], F32, name="gamma")
            beta = persist.tile([1, KG], F32, name="beta")
            warm16 = persist.tile([128, 16], F16, name="warm16")
            zrow = persist.tile([1, 4 * K], F16, name="zrow")
            c_eps = persist.tile([1, 1], F32, name="c_eps")
            c_lnk = persist.tile([1, 1], F32, name="c_lnk")
            ss = persist.tile([1, 3 * KG], F16, name="ss")
            stats_sb = persist.tile([1, 2 * KG], F32, name="stats_sb")
            stats_g = persist.tile([1, 2 * KG], F32, name="stats_g")
            bcB = persist.tile([128, 3 * KG], F16, name="bcB")

            stats_in = dram.tile([1, 2 * KG], F32, name="stats_in")
            stats_out = dram.tile([1, 2 * KG], F32, name="stats_out")

            # ---- phase 0: constants + x load/cast ----
            nc.vector.memset(warm16[:], 1.0)
            nc.vector.memset(zrow[:], 0.0)
            nc.vector.memset(ones16[:], 1.0)
            nc.vector.memset(ones_row_h[:], 1.0)
            nc.vector.memset(c_eps[:], BN_EPS)
            nc.vector.memset(c_lnk[:], float(-0.5 * np.log(K)))
            # early dummy activation: the single table load (1.3us) hides
            # under the x-load DMA phase
            nc.scalar.square(warm16[:], warm16[:])
            nc.sync.dma_start(identB[:], ident_c.ap()[:, :])
            nc.sync.dma_start(gamma[:], gam[:, :])
            nc.sync.dma_start(beta[:], bet[:, :])
            # x cast-DMA in groups of 4 token tiles (SWDGE casts fp32->fp16
            # in the DMA engines); clusters interleaved after the first two
            # groups so the first transposes start ASAP
            xr = x.ap().rearrange("(t p) d -> p t d", p=128)
            for g in range(2):
                nc.gpsimd.dma_start(
                    xh[:, 4 * g:4 * (g + 1), :], xr[:, 4 * g:4 * (g + 1), :])
            # clusters / clusters2 -> fp16 (cast dma): chunk c part p = row 128c+p
            nc.gpsimd.dma_start(
                clh[:], cl.ap().rearrange("(c p) k -> p c k", p=128))
            nc.gpsimd.dma_start(
                c2h[:], c2.ap().rearrange("(c p) k -> p c k", p=128))
            for g in range(2, nt // 4):
                nc.gpsimd.dma_start(
                    xh[:, 4 * g:4 * (g + 1), :], xr[:, 4 * g:4 * (g + 1), :])

            # ---- phase 1: PE transposes + assignment matmul + BN stats ----
            with tc.tile_pool(name="ps1", bufs=1, space="PSUM") as ps1:
                # every PSUM tag gets full-bank tiles so concurrent
                # accumulation groups never share a bank
                # one open accumulation group per PSUM bank at any time:
                # a start=True wipes other OPEN groups in its bank (probed
                # on hw); stopped data survives.
                xsum = ps1.tile([128, 8], F32, name="xsum", tag="xsum",
                                bufs=1)
                pstq = ps1.tile([1, 512], F32, name="pstq", tag="pstq",
                                bufs=1)

                # warm up the tensor engine p-state clock at t~0 (own
                # region of the pstq bank, single-shot)
                nc.tensor.matmul(pstq[0:1, 400:416], warm16[:, :1],
                                 warm16[:], start=True, stop=True,
                                 skip_group_check=True)
                # zero-start the xsum bank once; all per-tile column-sum
                # matmuls then accumulate start=False (one open group)
                nc.tensor.matmul(xsum[:, 0:4], ones_row_h[:], zrow[0:1, 0:4],
                                 start=True, stop=False,
                                 skip_group_check=True)

                # clusters2 transposed to (64k, 4c, 128d) for the fused
                # -a_sum*c2 accumulation into the vlad psum (borrows an
                # xT-pool tile before the pipeline starts)
                c2tp = ps1.tile([128, 2, 4, 128], F16, name="c2tp", tag="xT",
                                bufs=3)
                for c in range(4):
                    nc.tensor.transpose(c2tp[0:64, 0, c, :], c2h[:, c, :],
                                        ident)
                nc.vector.tensor_copy(c2hT[:], c2tp[0:64, 0, :, :])

                ng = nt // 4
                DEPTH = 4               # software pipeline depth (tiles)
                p1_groups = {}
                xhT_pairs = {}

                def pair_front(p):
                    """transposes + paired xT copy + xsum matmuls, tiles
                    t=2p, 2p+1."""
                    xp = ps1.tile([128, 2, 4, 128], F16, name="xp", tag="xT",
                                  bufs=3)
                    for j in range(2):
                        t = 2 * p + j
                        for c in range(4):
                            nc.tensor.transpose(
                                xp[:, j, c, :],
                                xh[:, t, 128 * c:128 * (c + 1)], ident)

                    for j in range(2):
                        t = 2 * p + j
                        for c in range(4):
                            nc.tensor.matmul(
                                xsum[:, c:c + 1],
                                xh[:, t, 128 * c:128 * (c + 1)],
                                ones16[:], start=False, stop=(t == nt - 1),
                                skip_group_check=True)
                    xt = work.tile([128, 2, 4, 128], F16, name="xhT",
                                   tag="xhT", bufs=4)
                    xhT_pairs[p] = xt
                    # PSUM->SBUF copies: ~25 pairs on DVE, 7 on Act
                    if p % 5 == 4:
                        nc.scalar.copy(xt[:], xp[:])
                    else:
                        nc.vector.tensor_copy(xt[:], xp[:])

                def tile_back(t):
                    """assignment matmuls for tile t + group stats."""
                    g = t // 4
                    if t % 4 == 0:
                        p1_groups[g] = ps1.tile([128, 4, 128], F32, name="p1",
                                                tag="p1", bufs=2)
                    p1g = p1_groups[g]
                    xt = xhT_pairs[t // 2]
                    for c in range(4):
                        nc.tensor.matmul(
                            p1g[:, t % 4, :KG], xt[:, t % 2, c, :],
                            clh[:, c, :], start=(c == 0), stop=(c == 3),
                            skip_group_check=True)
                    if t % 2 == 1:
                        xhT_pairs.pop(t // 2)
                    if t % 4 == 3:
                        # assn spill on Act; squares on DVE from the fp16
                        # copy (2x mode); column sums of squares on PE
                        nc.scalar.copy(assn[:, t - 3:t + 1, :],
                                       p1g[:, :, :KG])
                        asq = work.tile([128, 4, KG], F16, name="asq",
                                        tag="asq", bufs=3)
                        with nc.allow_low_precision("fp16 squares"):
                            nc.vector.tensor_tensor(
                                asq[:], assn[:, t - 3:t + 1, :],
                                assn[:, t - 3:t + 1, :], op=OP.mult)
                        nc.tensor.matmul(pstq[0:1, :4 * KG], ones16[:],
                                         asq[:], start=(g == 0),
                                         stop=(g == ng - 1),
                                         skip_group_check=True)
                        p1_groups.pop(g)

                for p in range(nt // 2):
                    pair_front(p)
                    for j in range(2):
                        t = 2 * p + j - DEPTH
                        if t >= 0:
                            tile_back(t)
                for t in range(nt - DEPTH, nt):
                    tile_back(t)

                # ---- phase 2 head: stats [sum(80) | sumsq(80)];
                # sum_n assn == (sum_n x) @ clusters by linearity ----
                nc.vector.tensor_copy(xsum_sb[:], xsum[:, :4])
                pj = pstq[0:1, 420:420 + KG]
                for c in range(4):
                    nc.tensor.matmul(pj, xsum_sb[:, c:c + 1], clh[:, c, :],
                                     start=(c == 0), stop=(c == 3),
                                     skip_group_check=True)
                nc.vector.tensor_copy(stats_sb[:, :KG], pj)
                nc.vector.tensor_reduce(
                    stats_sb[:, KG:],
                    pstq[0:1, :4 * KG].rearrange("p (t k) -> p k t", t=4),
                    axis=AX.X, op=OP.add)

            # ---- phase 2: all-reduce stats + BN parameters ----
            nc.sync.dma_start(stats_in[:], stats_sb[:])
            if with_collective:
                nc.gpsimd.collective_compute(
                    "AllReduce", OP.add,
                    replica_groups=[list(range(n_cores))],
                    ins=[stats_in.opt()], outs=[stats_out.opt()])
            else:
                nc.sync.dma_start(stats_out[:], stats_in[:])
            nc.sync.dma_start(stats_g[:], stats_out[:])

            t_mean = work.tile([1, KG], F32, name="t_mean", tag="sv", bufs=6)
            t_msq = work.tile([1, KG], F32, name="t_msq", tag="sv", bufs=6)
            t_var = work.tile([1, KG], F32, name="t_var", tag="sv", bufs=6)
            t_ln = work.tile([1, KG], F32, name="t_ln", tag="sv", bufs=6)
            t_rs = work.tile([1, KG], F32, name="t_rs", tag="sv", bufs=6)
            t_ms = work.tile([1, KG], F32, name="t_ms", tag="sv", bufs=6)
            inv_n = 1.0 / float(total_tok)
            nc.vector.tensor_scalar_mul(t_mean[:], stats_g[:, :KG], inv_n)
            nc.vector.tensor_tensor(t_msq[:], t_mean[:], t_mean[:], op=OP.mult)
            # var = sumsq*inv_n - mean^2 in one fused op
            nc.vector.scalar_tensor_tensor(
                t_var[:], stats_g[:, KG:], inv_n, t_msq[:],
                op0=OP.mult, op1=OP.subtract)
            # rsqrt(var+eps) = exp(-0.5*ln(var+eps)): stays in the ln/exp
            # activation table (no table reload vs Sqrt)
            nc.scalar.activation(t_ln[:], t_var[:], ACTF.Ln, bias=c_eps[:])
            nc.scalar.activation(t_rs[:], t_ln[:], ACTF.Exp, scale=-0.5)
            nc.vector.tensor_tensor(ss[:, :KG], t_rs[:], gamma[:], op=OP.mult)
            nc.vector.tensor_tensor(t_ms[:], t_mean[:], ss[:, :KG], op=OP.mult)
            nc.vector.tensor_tensor(ss[:, KG:2 * KG], beta[:], t_ms[:],
                                    op=OP.subtract)
            # w = exp(shift): folding the BN shift into the denominator
            # weights (it cancels everywhere else after normalization)
            with nc.allow_low_precision("fp16 softmax weights"):
                nc.scalar.activation(ss[:, 2 * KG:], ss[:, KG:2 * KG],
                                     ACTF.Exp)

            # ---- phases 3-5: softmax, vlad matmul, normalization ----
            # Half-batch (8-tile) softmax granularity keeps every in-order
            # engine queue busy and lets the vlad matmuls start early.
            H = TPB // 2
            with (
                tc.tile_pool(name="ps2", bufs=1, space="PSUM") as ps2,
                tc.tile_pool(name="elem", bufs=3) as elem,
                tc.tile_pool(name="vpost", bufs=2) as vpost,
            ):
                pbc = ps2.tile([128, 3 * KG], F32, name="pbc", tag="pbc",
                               bufs=1)
                # scale+shift broadcast gates te0; the w row (needs an extra
                # Act exp) follows separately off the critical path
                nc.tensor.matmul(pbc[:, :2 * KG], ones_row_h[:],
                                 ss[:, :2 * KG], start=True, stop=True,
                                 skip_group_check=True)
                nc.vector.tensor_copy(bcB[:, :2 * KG], pbc[:, :2 * KG])
                nc.tensor.matmul(pbc[:, 2 * KG:], ones_row_h[:],
                                 ss[:, 2 * KG:], start=True, stop=True,
                                 skip_group_check=True)
                nc.scalar.copy(bcB[:, 2 * KG:], pbc[:, 2 * KG:])
                scale_b = bcB[:, :KG].rearrange("p (a k) -> p a k", a=1)
                shift_b = bcB[:, KG:2 * KG].rearrange("p (a k) -> p a k", a=1)
                w_b = bcB[:, 2 * KG:].rearrange("p (a k) -> p a k", a=1)

                te_tiles = {}
                state = {}
                # engine per batch: 'v' = DVE, 'p' = gpsimd (stt ops)
                TE_ENG = {0: 'v', 1: 'p', 2: 'v', 3: 'p'}
                SM_ENG = {0: 'p', 1: 'p', 2: 'v', 3: 'p'}

                def softmax_head(b, h):
                    """BN-apply (DVE or Pool) + exp (Act) for half h."""
                    t0 = b * TPB + h * H
                    te = elem.tile([128, H, KG], F16, name="te", tag="te",
                                   bufs=5)
                    te_tiles[(b, h)] = te
                    if TE_ENG[b] == 'v':
                        nc.vector.tensor_tensor(
                            te[:], assn[:, t0:t0 + H, :],
                            scale_b.to_broadcast([128, H, KG]), op=OP.mult)
                        nc.vector.tensor_tensor(
                            te[:], te[:],
                            shift_b.to_broadcast([128, H, KG]), op=OP.add)
                    else:
                        nc.gpsimd.tensor_tensor(
                            te[:], assn[:, t0:t0 + H, :],
                            scale_b.to_broadcast([128, H, KG]), op=OP.mult)
                    nc.scalar.activation(te[:], te[:], ACTF.Exp)

                def softmax_tail(b, h):
                    """denominator fold+reduce, recip, sm for half h."""
                    t0 = b * TPB + h * H
                    te = te_tiles.pop((b, h))
                    if TE_ENG[b] == 'p':
                        # denominator needs the e^shift weights the Pool
                        # path skipped
                        ew = elem.tile([128, H, KG], F16, name="ew",
                                       tag="ew", bufs=2)
                        nc.vector.tensor_tensor(
                            ew[:], te[:],
                            w_b.to_broadcast([128, H, KG]), op=OP.mult)
                        fsrc = ew
                    else:
                        fsrc = te
                    f1 = elem.tile([128, H, KG // 2], F16, name="f1",
                                   tag="f1", bufs=2)
                    with nc.allow_low_precision("fp16 softmax denom"):
                        nc.vector.tensor_tensor(
                            f1[:], fsrc[:, :, :KG // 2],
                            fsrc[:, :, KG // 2:], op=OP.add)
                        denom = work.tile([128, H], F16, name="denom",
                                          tag="dn", bufs=2)
                        nc.vector.tensor_reduce(denom[:], f1[:], axis=AX.X,
                                                op=OP.add)
                        recip = work.tile([128, H], F16, name="recip",
                                          tag="rc", bufs=2)
                        nc.vector.reciprocal(recip[:], denom[:])
                    rb = recip[:].rearrange("p (t a) -> p t a", a=1) \
                        .to_broadcast([128, H, K])
                    if SM_ENG[b] == 'v':
                        nc.vector.tensor_tensor(
                            sm[:, t0:t0 + H, :], te[:, :, :K], rb, op=OP.mult)
                    else:
                        nc.gpsimd.tensor_tensor(
                            sm[:, t0:t0 + H, :], te[:, :, :K], rb, op=OP.mult)

                def mm_seg(b, h):
                    """vlad + a_sum matmul segment for half h; the c-groups
                    stay open until the -a_sum*c2 closer in post_head."""
                    t0 = b * TPB + h * H
                    if h == 0:
                        pv = ps2.tile([128, 4, 128], F32, name="pv", tag="pv",
                                      bufs=3)
                        ppost = ps2.tile([1, 512], F32, name="ppost",
                                         tag="ppost", bufs=2)
                        state[b] = (pv, ppost)
                        # zero the whole bank region with one start matmul
                        # so ONE group stays open; everything after
                        # accumulates with start=False
                        nc.tensor.matmul(
                            pv[:, :, :K], ones_row_h[:], zrow[:],
                            start=True, stop=False, skip_group_check=True)
                    pv, ppost = state[b]
                    pas = ppost[0:1, :K]
                    for c in range(4):
                        for i in range(H):
                            t = t0 + i
                            nc.tensor.matmul(
                                pv[:, c, :K],
                                xh[:, t, c * 128:(c + 1) * 128],
                                sm[:, t, :],
                                start=False, stop=False,
                                skip_group_check=True)
                    for i in range(H):
                        nc.tensor.matmul(pas, ones16[:], sm[:, t0 + i, :],
                                         start=(h == 0 and i == 0),
                                         stop=(h == 1 and i == H - 1),
                                         skip_group_check=True)

                def post_head(b):
                    """a_sum bcast, -a*c2 folded into the vlad psum via a
                    negated-identity matmul, squares, intra-norm sums,
                    rn = exp(-0.5 ln(nrm2) + ln(1/sqrt(K)))."""
                    pv, ppost = state.pop(b)
                    pa_sb = work.tile([1, K], F16, name="pa_sb", tag="pas_sb",
                                      bufs=2)
                    with nc.allow_low_precision("fp16 a_sum"):
                        nc.vector.tensor_copy(pa_sb[:], ppost[0:1, :K])
                    bc = ps2.tile([128, 2 * K], F32, name="bc", tag="bc",
                                  bufs=2)
                    pamB = bc[:, :K]
                    nc.tensor.matmul(pamB, ones_row_h[:], pa_sb[:],
                                     start=True, stop=True,
                                     skip_group_check=True)
                    dgN = vpost.tile([64, K], F16, name="dgN", tag="dgN")
                    nc.vector.tensor_tensor(dgN[:], identN, pamB[0:64, :],
                                            op=OP.mult)
                    # close the four c-groups: pv[:,c,:] -= c2[:,k]*a_sum[k]
                    for c in range(4):
                        nc.tensor.matmul(pv[:, c, :K], c2hT[:, c, :], dgN[:],
                                         start=False, stop=True,
                                         skip_group_check=True)
                    v = vpost.tile([128, 4, K], F16, name="v", tag="v")
                    nc.scalar.copy(v[:], pv[:, :, :K])
                    sq = vpost.tile([128, 4, K], F16, name="sq", tag="sq")
                    with nc.allow_low_precision("fp16 norm squares"):
                        nc.vector.tensor_tensor(sq[:], v[:], v[:], op=OP.mult)
                    pnrm = ppost[0:1, K:2 * K]
                    for c in range(4):
                        nc.tensor.matmul(pnrm, ones16[:], sq[:, c, :],
                                         start=(c == 0), stop=(c == 3),
                                         skip_group_check=True)
                    # global L2 norm is exactly sqrt(K) (K unit columns),
                    # folded in via the exp bias
                    rn = work.tile([1, K], F16, name="rn", tag="nr", bufs=4)
                    lnn = work.tile([1, K], F32, name="lnn", tag="nr2",
                                    bufs=4)
                    nc.scalar.activation(lnn[:], pnrm, ACTF.Ln, bias=0.0)
                    with nc.allow_low_precision("fp16 norm recip"):
                        nc.scalar.activation(rn[:], lnn[:], ACTF.Exp,
                                             scale=-0.5, bias=c_lnk[:])
                    state[b] = (v, rn, bc)

                def post_tail(b):
                    v, rn, bc = state.pop(b)
                    prnB = bc[:, K:]
                    nc.tensor.matmul(prnB, ones_row_h[:], rn[:],
                                     start=True, stop=True,
                                     skip_group_check=True)
                    vf = vpost.tile([128, 4, K], F32, name="vf", tag="vf")
                    nc.vector.tensor_tensor(
                        vf[:], v[:],
                        prnB.rearrange("p (a k) -> p a k", a=1)
                        .to_broadcast([128, 4, K]), op=OP.mult)
                    yb = y[b, :].rearrange("(c p k) -> p c k", p=128, k=K)
                    nc.sync.dma_start(yb[:, :, :], vf[:])

                # emission: batch 0/2 prepared on DVE, 1/3 on gpsimd; posts
                # pipelined two batches behind
                softmax_head(0, 0)
                softmax_head(0, 1)
                softmax_head(1, 0)
                softmax_head(1, 1)
                for b in range(b_loc):
                    softmax_tail(b, 0)
                    mm_seg(b, 0)
                    if b >= 1:
                        post_head(b - 1)
                    softmax_tail(b, 1)
                    mm_seg(b, 1)
                    if b + 2 < b_loc:
                        softmax_head(b + 2, 0)
                        softmax_head(b + 2, 1)
                    if b >= 2:
                        post_tail(b - 2)
                post_head(b_loc - 1)
                post_tail(b_loc - 2)
                post_tail(b_loc - 1)
    tabs, saved = _steer_act_tables(nc)
    try:
        nc.compile()
    finally:
        _restore_act_tables(tabs, saved)
    return nc


_CACHE = {}


def _get(b_loc, n_cores, with_collective):
    key = (b_loc, n_cores, with_collective)
    if key not in _CACHE:
        _CACHE[key] = build(b_loc, n_cores, with_collective)
    return _CACHE[key]


def make_in_maps(x, clusters, clusters2, bn_gamma, bn_beta, n_cores=N_CORES):
    B = x.shape[0]
    b_loc = B // n_cores
    shared = {
        "clusters": np.ascontiguousarray(clusters, np.float32),
        "clusters2": np.ascontiguousarray(
            np.asarray(clusters2).reshape(D, K), np.float32),
        "bn_gamma": np.ascontiguousarray(
            np.asarray(bn_gamma).reshape(1, KG), np.float32),
        "bn_beta": np.ascontiguousarray(
            np.asarray(bn_beta).reshape(1, KG), np.float32),
    }
    in_maps = []
    for i in range(n_cores):
        m = dict(shared)
        m["x"] = np.ascontiguousarray(
            np.asarray(x[i * b_loc:(i + 1) * b_loc]).reshape(
                b_loc * N_SEQ, D), np.float32)
        in_maps.append(m)
    return in_maps


def kernel(x, clusters, clusters2, bn_gamma, bn_beta):
    B, N, Dd = x.shape
    assert (N, Dd) == (N_SEQ, D) and B % N_CORES == 0
    b_loc = B // N_CORES
    nc = _get(b_loc, N_CORES, True)
    in_maps = make_in_maps(x, clusters, clusters2, bn_gamma, bn_beta)
    res = run_bass_kernel_spmd(nc, in_maps, core_ids=list(range(N_CORES)))
    out = np.concatenate([res.results[i]["y"] for i in range(N_CORES)], axis=0)
    return out
